# revision 1
# baseline (speedup 1.0000x reference)
"""Trainium2 Bass kernel for nn_ErrorAwareSelfAttention (8 NeuronCores).

Design (see inline notes):
- Stage A sharded by 8-image-row strips on cores 0-5 (window-aligned):
  k^T (ch-major) + v (pixel-major) projections, Modulator convs on frames
  0,1 (1-px halo from host-padded x_strip), pooled tokens sharded over all
  8 cores. One AllGather publishes k^T / v / pk^T / pv to every core.
- Stage B sharded 5-padded-windows per core: global attention per
  (window, head) with keys = 16x16 halo patch (the rolled+valid_ind key
  set is exactly that patch; attention is permutation-invariant over keys)
  + 576 pooled keys; local per-frame attention over the own 8x8 window;
  blend by the per-window mask flag; output projection; host scatters.
- Scores stay transposed (keys on partitions, 256 queries on free) so the
  softmax exp is a single ACT pass PSUM->SBUF and AV consumes p^T
  directly. Softmax denominators via PE-transpose + DVE row reduce; no
  max-subtraction (scores are O(1) by construction: 0.02-scale weights).
- All matmuls fp32 (4 cyc/row; float32r is broken on this platform: its
  DMA rounding contaminates unrelated transfers).
"""

import math
import sys

sys.path.insert(0, "/opt/trn_rl_repo")

import numpy as np

import concourse.bass as bass
import concourse.mybir as mybir
import concourse.tile as tile
from concourse import bacc
from concourse.bass_utils import run_bass_kernel_spmd
from concourse.masks import make_identity

dt = mybir.dt
AF = mybir.ActivationFunctionType
AX = mybir.AxisListType

# ---------------- problem constants (hardcoded) ----------------
DIM = 512
N_HEAD = 4
CH = 128
WH = WW = 8
EH = EW = 4
PH = PW = 4
B, T, HI, WI = 1, 4, 48, 48
L_T = 2
N_WH = N_WW = 6
NW = 36
WN = 64
SCALE = 1.0 / math.sqrt(CH)
N_CORES = 8
NC4 = 4  # 512 / 128 channel chunks

# stage A strips: 6 strips of 8 image rows (cores 0-5). x_strip has 1-row halo.
STRIP_H = 8
N_STRIP = 6
# stage B: 5 windows per core (padded; 36 windows total)
WPC = 5
_bounds = [int(NW * c / N_CORES) for c in range(N_CORES + 1)]
WIN_ASSIGN = []  # per core: list of 5 window ids (last repeated as padding)
for c in range(N_CORES):
    ws = list(range(_bounds[c], _bounds[c + 1]))
    while len(ws) < WPC:
        ws.append(ws[-1])
    WIN_ASSIGN.append(ws)

# pooled tokens: 12x12 per frame -> 576 rows, 72 per core
PGRID = HI // PH  # 12
NPOOL = T * PGRID * PGRID  # 576
POOL_PC = NPOOL // N_CORES  # 72

NPIX = T * HI * WI  # 9216
PATCH_PIX = 4 * 16 * 16  # 1024 keys/window from the halo patch (4 frames)
NKEYS = PATCH_PIX + NPOOL  # 1600 global keys
NQ = T * WN  # 256 queries per window
KCH = [128] * 8 + [POOL_PC] * 8  # 16 key chunks: 8 patch + 8 pooled(72)
NCHUNK = len(KCH)
# flat AllGather packing offsets (f32 elements)
OFF_K = 0
OFF_V = OFF_K + DIM * T * STRIP_H * 64
OFF_PK = OFF_V + T * STRIP_H * 64 * DIM
OFF_PV = OFF_PK + DIM * POOL_PC
AG_TOT = OFF_PV + POOL_PC * DIM

_NC_CACHE = {}


def _meta_for_core(c):
    """Per-window dynamic DMA registers: slot_prev, slot_cur, slot_next,
    x0 (patch x start in the 64-wide margin layout), x_own (=8j)."""
    vals = []
    for w in WIN_ASSIGN[c]:
        i, j = w // N_WW, w % N_WW
        vals += [(i - 1) % N_STRIP, i, (i + 1) % N_STRIP, (8 * j - 4) % 48, 8 * j]
    vals += [0] * (32 - len(vals) % 32 if len(vals) % 32 else 0)
    return np.asarray(vals[: ((len(vals) + 31) // 32) * 32], np.int32)


META_LEN = len(_meta_for_core(0))


def build_nc(debug=False):
    nc = bacc.Bacc("TRN2", target_bir_lowering=False, debug=True)

    # ---------------- I/O ----------------
    x_strip = nc.dram_tensor("x_strip", [T, STRIP_H + 2, WI, DIM], dt.float32,
                             kind="ExternalInput")
    x_win = nc.dram_tensor("x_win", [WPC, T, WN, DIM], dt.float32,
                           kind="ExternalInput")
    x_pool = nc.dram_tensor("x_pool", [POOL_PC // PGRID * PH, WI, DIM],
                            dt.float32, kind="ExternalInput")  # [24,48,512]
    mask_strip = nc.dram_tensor("mask_strip", [L_T, STRIP_H + 2, WI],
                                dt.float32, kind="ExternalInput")
    mask_win = nc.dram_tensor("mask_win", [L_T, WPC, WN], dt.float32,
                              kind="ExternalInput")
    halo_scale = nc.dram_tensor("halo_scale", [2], dt.float32,
                                kind="ExternalInput")
    meta = nc.dram_tensor("meta", [META_LEN], dt.int32, kind="ExternalInput")
    pool_ind = nc.dram_tensor("pool_ind", [2 * WI, PGRID], dt.float32,
                              kind="ExternalInput")  # [96,12] avg indicator

    wq_d = nc.dram_tensor("wq", [DIM, DIM], dt.float32, kind="ExternalInput")
    wk_d = nc.dram_tensor("wk", [DIM, DIM], dt.float32, kind="ExternalInput")
    wv_d = nc.dram_tensor("wv", [DIM, DIM], dt.float32, kind="ExternalInput")
    wp_d = nc.dram_tensor("wp", [DIM, DIM], dt.float32, kind="ExternalInput")
    bq_d = nc.dram_tensor("bq", [DIM], dt.float32, kind="ExternalInput")
    bk_d = nc.dram_tensor("bk", [DIM], dt.float32, kind="ExternalInput")
    bv_d = nc.dram_tensor("bv", [DIM], dt.float32, kind="ExternalInput")
    bp_d = nc.dram_tensor("bp", [DIM], dt.float32, kind="ExternalInput")
    pool_b_d = nc.dram_tensor("pool_b", [DIM], dt.float32, kind="ExternalInput")
    # modulator weights, host pre-transposed to [in,out]
    mods = {}
    for tag in ("k", "v"):
        mods[tag] = dict(
            sq=nc.dram_tensor(f"{tag}sq", [DIM, 128], dt.float32, kind="ExternalInput"),
            sqb=nc.dram_tensor(f"{tag}sqb", [128], dt.float32, kind="ExternalInput"),
            f=nc.dram_tensor(f"{tag}f", [9, 128, 128], dt.float32, kind="ExternalInput"),
            fb=nc.dram_tensor(f"{tag}fb", [128], dt.float32, kind="ExternalInput"),
            un=nc.dram_tensor(f"{tag}un", [128, DIM], dt.float32, kind="ExternalInput"),
            unb=nc.dram_tensor(f"{tag}unb", [DIM], dt.float32, kind="ExternalInput"),
        )

    out_win = nc.dram_tensor("out_win", [WPC, T, WN, DIM], dt.float32,
                             kind="ExternalOutput")
    dbg = {}
    if debug:
        dbg["k_contrib"] = nc.dram_tensor("dbg_k", [DIM, T, STRIP_H, 64],
                                          dt.float32, kind="ExternalOutput")
        dbg["v_contrib"] = nc.dram_tensor("dbg_v", [T, STRIP_H, 64, DIM],
                                          dt.float32, kind="ExternalOutput")
        dbg["pk"] = nc.dram_tensor("dbg_pk", [DIM, POOL_PC], dt.float32,
                                   kind="ExternalOutput")
        dbg["pv"] = nc.dram_tensor("dbg_pv", [POOL_PC, DIM], dt.float32,
                                   kind="ExternalOutput")
        dbg["q"] = nc.dram_tensor("dbg_q", [NC4, 128, WPC * NQ], dt.float32,
                                  kind="ExternalOutput")
        dbg["flags"] = nc.dram_tensor("dbg_flags", [WPC], dt.float32,
                                      kind="ExternalOutput")

    # internal DRAM for collective (single flat packed buffer)
    contrib = nc.dram_tensor("contrib", [1, AG_TOT], dt.float32)
    agout = nc.dram_tensor("agout", [N_CORES, AG_TOT], dt.float32,
                           addr_space="Shared")
    flags_d = nc.dram_tensor("flags_d", [WPC], dt.float32)

    with tile.TileContext(nc, num_cores=N_CORES) as tc:
        _program(nc, tc, locals())
    nc.compile()
    return nc


def _program(nc, tc, g):
    with (
        tc.tile_pool(name="consts", bufs=1) as consts,
        tc.tile_pool(name="wpool", bufs=1) as wpool,
        tc.tile_pool(name="ps_big", bufs=2, space="PSUM") as ps_big,
        tc.tile_pool(name="ps_mid", bufs=4, space="PSUM") as ps_mid,
        tc.tile_pool(name="ps_y", bufs=2, space="PSUM") as ps_y,
    ):
        ident = consts.tile([128, 128], dt.float32)
        make_identity(nc, ident)

        # weights as [128 (in chunk), 4 in-chunks, 512 out]
        W = {}
        for nm in ("wq", "wk", "wv", "wp"):
            t = wpool.tile([128, NC4, DIM], dt.float32, tag=nm)
            nc.sync.dma_start(t, g[nm + "_d"][:, :].rearrange("(a p) o -> p a o", p=128))
            W[nm] = t
        # per-partition bias tiles [128,1] x4 chunks: store as [128, 4]
        Bp = {}
        for nm in ("bq", "bk", "bv"):
            t = wpool.tile([128, NC4], dt.float32, tag=nm + "p")
            nc.sync.dma_start(t, g[nm + "_d"][:].rearrange("(a p) -> p a", p=128))
            Bp[nm] = t
        # free-axis broadcast bias tiles [128, 512]
        Bf = {}
        for nm in ("bv", "bp", "pool_b"):
            t = wpool.tile([128, DIM], dt.float32, tag=nm + "f")
            nc.sync.dma_start(t, g[nm + "_d"][:].unsqueeze(0).to_broadcast([128, DIM]))
            Bf[nm] = t
        MW = {}
        for tag in ("k", "v"):
            m = g["mods"][tag]
            MW[tag] = dict(
                sq=wpool.tile([128, NC4, 128], dt.float32, tag=f"{tag}sqw", name=f"{tag}sqw"),
                sqb=wpool.tile([128, 1], dt.float32, tag=f"{tag}sqbw", name=f"{tag}sqbw"),
                f=wpool.tile([128, 9, 128], dt.float32, tag=f"{tag}fw", name=f"{tag}fw"),
                fb=wpool.tile([128, 1], dt.float32, tag=f"{tag}fbw", name=f"{tag}fbw"),
                un=wpool.tile([128, DIM], dt.float32, tag=f"{tag}unw", name=f"{tag}unw"),
                unb=wpool.tile([128, NC4], dt.float32, tag=f"{tag}unbw", name=f"{tag}unbw"),
            )
            nc.sync.dma_start(MW[tag]["sq"], m["sq"][:, :].rearrange("(a p) o -> p a o", p=128))
            nc.sync.dma_start(MW[tag]["sqb"], m["sqb"][:].unsqueeze(1))
            nc.sync.dma_start(MW[tag]["f"], m["f"][:, :, :].rearrange("n p o -> p n o"))
            nc.sync.dma_start(MW[tag]["fb"], m["fb"][:].unsqueeze(1))
            nc.sync.dma_start(MW[tag]["un"], m["un"][:, :])
            nc.sync.dma_start(MW[tag]["unb"], m["unb"][:].rearrange("(a p) -> p a", p=128))
        pool_ind_t = wpool.tile([2 * WI, PGRID], dt.float32)
        nc.sync.dma_start(pool_ind_t, g["pool_ind"][:, :])
        hs = wpool.tile([128, 2], dt.float32)
        nc.sync.dma_start(hs, g["halo_scale"][:].unsqueeze(0).to_broadcast([128, 2]))

        # ---------------- stage B prologue: x_win^T, q^T (no AG dep) -------
        with tc.tile_pool(name="sbB0", bufs=1) as sbB0:
            # ================= stage A =================
            with tc.tile_pool(name="sbA", bufs=1) as sbA:
                _stage_a(nc, tc, g, sbA, ps_big, ps_mid, W, Bp, Bf, MW,
                         pool_ind_t, hs, ident)

            qT = sbB0.tile([128, NC4, WPC * NQ], dt.float32)
            with tc.tile_pool(name="sbXW", bufs=1) as sbXW:
                xtw = sbXW.tile([128, NC4, WPC * NQ], dt.float32)
                _transpose_in(nc, ps_big, sbXW,
                              g["x_win"][:, :, :, :].rearrange("w t p c -> (w t p) c"),
                              xtw, WPC * NQ, ident)
                for oc in range(NC4):
                    for piece in range(4):
                        s = piece * 320
                        ps = ps_big.tile([128, 320], dt.float32, tag="big")
                        for ic in range(NC4):
                            nc.tensor.matmul(ps, W["wq"][:, ic, oc * 128:(oc + 1) * 128],
                                             xtw[:, ic, s:s + 320],
                                             start=(ic == 0), stop=(ic == NC4 - 1))
                        nc.scalar.activation(qT[:, oc, s:s + 320], ps, AF.Identity,
                                             bias=Bp["bq"][:, oc:oc + 1])
            if "q" in g["dbg"]:
                for oc in range(NC4):
                    nc.sync.dma_start(g["dbg"]["q"][oc, :, :], qT[:, oc, :])

            # ---------------- flags ----------------
            mwt = sbB0.tile([L_T, WPC * WN], dt.float32)
            nc.sync.dma_start(mwt, g["mask_win"][:, :, :].rearrange("l w p -> l (w p)"))
            mx = sbB0.tile([L_T, WPC, 1], dt.float32)
            nc.vector.reduce_max(mx, mwt.rearrange("l (w p) -> l w p", w=WPC),
                                 axis=AX.X, opt_input=False, opt_output=False)
            with tc.tile_pool(name="flg_d", bufs=1, space="DRAM") as flgp:
                mx_d = flgp.tile([L_T, WPC], dt.float32)
                nc.sync.dma_start(mx_d, mx[:, :, 0])
                mrow = sbB0.tile([1, L_T * WPC], dt.float32)
                nc.sync.dma_start(mrow, mx_d[:, :].rearrange("l w -> (l w)")
                                  .unsqueeze(0))
            msum = sbB0.tile([1, WPC], dt.float32)
            nc.vector.tensor_add(msum, mrow[:, 0:WPC], mrow[:, WPC:2 * WPC])
            fl = sbB0.tile([1, WPC], dt.float32)
            nc.scalar.activation(fl, msum, AF.Sign)
            nc.sync.dma_start(g["flags_d"][:].unsqueeze(0), fl[0:1, :])
            if "flags" in g["dbg"]:
                nc.sync.dma_start(g["dbg"]["flags"][:].unsqueeze(0), fl[0:1, :])

            # ---------------- AllGather ----------------
            nc.gpsimd.collective_compute(
                "AllGather", mybir.AluOpType.bypass,
                ins=[g["contrib"][:, :]],
                outs=[g["agout"][:, :]],
                replica_groups=[list(range(N_CORES))],
            )

            # ================= stage B =================
            _stage_b(nc, tc, g, sbB0, ps_big, ps_mid, ps_y, W, Bp, Bf, qT, ident)


def _transpose_in(nc, ps_pool, sb_pool, src_ap, dst, npix, ident):
    """DMA pixel-major [npix, 512] DRAM -> transposed SBUF [128, 4, npix]."""
    for t in range((npix + 127) // 128):
        n = min(128, npix - t * 128)
        tmp = sb_pool.tile([128, DIM], dt.float32, tag="tr_in")
        nc.sync.dma_start(tmp[0:n, :], src_ap[t * 128:t * 128 + n, :])
        ps = ps_pool.tile([128, DIM], dt.float32, tag="big")
        for ic in range(NC4):
            nc.tensor.transpose(ps[:, ic * 128:ic * 128 + n],
                                tmp[0:n, ic * 128:(ic + 1) * 128], ident[0:n, 0:n])
        for ic in range(NC4):
            nc.vector.tensor_copy(dst[:, ic, t * 128:t * 128 + n],
                                  ps[:, ic * 128:ic * 128 + n])


def _stage_a(nc, tc, g, sb, ps_big, ps_mid, W, Bp, Bf, MW, pool_ind_t, hs, ident):
    HP = STRIP_H + 2  # 10 rows incl halo
    PIX01 = 2 * HP * WI  # 960 f01 pixels (with halo rows)
    PIX23 = 2 * STRIP_H * WI  # 768

    xs = g["x_strip"]
    maskb = sb.tile([128, PIX01], dt.float32)
    nc.sync.dma_start(maskb, g["mask_strip"][:, :, :].rearrange("l y x -> (l y x)")
                      .unsqueeze(0).to_broadcast([128, PIX01]))

    kT23 = sb.tile([128, NC4, PIX23], dt.float32)
    kmod_in = sb.tile([128, NC4, PIX01], dt.float32)  # k01+mask (mod input)
    vmod_in = sb.tile([128, NC4, PIX01], dt.float32)
    vc = g["contrib"][0, OFF_V:OFF_PK].rearrange("(t y x c) -> t y x c",
                                                 t=T, y=STRIP_H, x=64)

    def _v_out(vt, fr, t):
        f, y = divmod(t, 4)  # (frame offset, 2-row group)
        nc.sync.dma_start(vc[fr + f, 2 * y:2 * y + 2, 0:48, :], vt)
        nc.sync.dma_start(vc[fr + f, 2 * y, 48:64, :], vt[0:16, :])
        nc.sync.dma_start(vc[fr + f, 2 * y + 1, 48:64, :], vt[48:64, :])

    with tc.tile_pool(name="sbXT", bufs=1) as sbXT:
        # x^T over strip: f01 all 10 rows, f23 middle 8 rows
        xt01 = sbXT.tile([128, NC4, PIX01], dt.float32, tag="xt")
        _transpose_in(nc, ps_big, sbXT, xs[0:2].rearrange("t y x c -> (t y x) c"),
                      xt01, PIX01, ident)
        # k/v f01 projections straight into modulator-input tiles (+mask)
        _proj_T(nc, ps_big, W["wk"], Bp["bk"], xt01, kmod_in, PIX01, 320)
        _proj_T(nc, ps_big, W["wv"], Bp["bv"], xt01, vmod_in, PIX01, 320)
        for ic in range(NC4):
            nc.vector.tensor_add(kmod_in[:, ic, :], kmod_in[:, ic, :], maskb)
            nc.vector.tensor_add(vmod_in[:, ic, :], vmod_in[:, ic, :], maskb)

        xt23 = sbXT.tile([128, NC4, PIX01], dt.float32, tag="xt")
        _transpose_in(nc, ps_big, sbXT,
                      xs[2:4].rearrange("t y x c -> (t y x) c"),
                      xt23, PIX01, ident)
        for f in range(2):
            _proj_T(nc, ps_big, W["wk"], Bp["bk"],
                    xt23[:, :, f * 480 + 48:f * 480 + 432],
                    kT23[:, :, f * 384:(f + 1) * 384], 384, 384)
        for t in range(8):
            f, grp = t // 4, t % 4
            s = f * 480 + 48 + grp * 96
            ps = ps_big.tile([96, DIM], dt.float32, tag="big")
            for ic in range(NC4):
                nc.tensor.matmul(ps, xt23[:, ic, s:s + 96],
                                 W["wv"][:, ic, :], start=(ic == 0),
                                 stop=(ic == NC4 - 1))
            vst = sb.tile([96, DIM], dt.float32, tag="vst", name="vst")
            nc.vector.tensor_add(vst, ps, Bf["bv"][0:96, :])
            _v_out(vst, 2, t)

    # ---- modulators (replace f01) ----
    kT01m = _modulator(nc, tc, sb, ps_big, ps_mid, MW["k"], kmod_in, hs, "k")
    vT01m = _modulator(nc, tc, sb, ps_big, ps_mid, MW["v"], vmod_in, hs, "v")

    # ---- transpose v01m back to natural 96-pix tiles, stream out ----
    vT01f = vT01m.rearrange("p a t y x -> p a (t y x)")
    for t in range(8):  # (f, 2-row group)
        s = t * 96
        ps = ps_big.tile([96, DIM], dt.float32, tag="big")
        for ic in range(NC4):
            nc.tensor.transpose(ps[:, ic * 128:(ic + 1) * 128],
                                vT01f[:, ic, s:s + 96], ident)
        vst = sb.tile([96, DIM], dt.float32, tag="vst", name="vst")
        nc.vector.tensor_copy(vst, ps)
        _v_out(vst, 0, t)

    # ---- AG contributions ----
    kc = g["contrib"][0, OFF_K:OFF_V].rearrange("(a p t y x) -> p a t y x", a=NC4, p=128, t=T, y=STRIP_H)
    k01v = kT01m  # [128, 4, 2, 8, 48]
    k23v = kT23.rearrange("p a (t y x) -> p a t y x", t=2, y=STRIP_H)
    for ic in range(NC4):
        nc.sync.dma_start(kc[:, ic, 0:2, :, 0:48], k01v[:, ic])
        nc.sync.dma_start(kc[:, ic, 2:4, :, 0:48], k23v[:, ic])
        nc.sync.dma_start(kc[:, ic, 0:2, :, 48:64], k01v[:, ic, :, :, 0:16])
        nc.sync.dma_start(kc[:, ic, 2:4, :, 48:64], k23v[:, ic, :, :, 0:16])
    if "k_contrib" in g["dbg"]:
        nc.sync.dma_start(
            g["dbg"]["k_contrib"][:, :, :, :].rearrange("d t y x -> (d t y x)"),
            g["contrib"][0, OFF_K:OFF_V])
        nc.sync.dma_start(
            g["dbg"]["v_contrib"][:, :, :, :].rearrange("t y x d -> (t y x d)"),
            g["contrib"][0, OFF_V:OFF_PK])

    # ---- pooled tokens (72 rows on every core) ----
    px = sb.tile([PGRID, 6, DIM], dt.float32)  # [cell 12, cell-row 6, ch]
    for cr in range(6):
        ps = ps_big.tile([PGRID, DIM], dt.float32, tag="big")
        for h in range(2):  # two 2-row groups of the 4-row cell
            xrows = sb.tile([96, DIM], dt.float32, tag="xpoolrows")
            r0 = cr * 4 + h * 2
            nc.sync.dma_start(xrows, g["x_pool"][r0:r0 + 2].rearrange("y x c -> (y x) c"))
            nc.tensor.matmul(ps, pool_ind_t, xrows, start=(h == 0), stop=(h == 1))
        nc.vector.tensor_add(px[:, cr, :], ps, Bf["pool_b"][0:PGRID, :])
    # px^T [128,4,72]  (pooled-row order = (cell-row, cell), cr-major)
    pxT = sb.tile([128, NC4, POOL_PC], dt.float32)
    for ic in range(NC4):
        ps = ps_mid.tile([128, POOL_PC], dt.float32, tag="mid")
        for cr in range(6):
            nc.tensor.transpose(ps[:, cr * PGRID:(cr + 1) * PGRID],
                                px[:, cr, ic * 128:(ic + 1) * 128],
                                ident[0:PGRID, 0:PGRID])
        nc.vector.tensor_copy(pxT[:, ic, :], ps)
    # pk^T
    pkT = sb.tile([128, NC4, POOL_PC], dt.float32)
    for oc in range(NC4):
        ps2 = ps_mid.tile([128, POOL_PC], dt.float32, tag="mid")
        for ic in range(NC4):
            nc.tensor.matmul(ps2, W["wk"][:, ic, oc * 128:(oc + 1) * 128],
                             pxT[:, ic, :], start=(ic == 0), stop=(ic == NC4 - 1))
        nc.scalar.activation(pkT[:, oc, :], ps2, AF.Identity, bias=Bp["bk"][:, oc:oc + 1])
    nc.sync.dma_start(g["contrib"][0, OFF_PK:OFF_PV].rearrange("(a p n) -> p a n", a=NC4, p=128), pkT)
    # pv natural
    pv = sb.tile([POOL_PC, DIM], dt.float32)
    ps3 = ps_big.tile([POOL_PC, DIM], dt.float32, tag="big")
    for ic in range(NC4):
        nc.tensor.matmul(ps3, pxT[:, ic, 0:POOL_PC], W["wv"][:, ic, :],
                         start=(ic == 0), stop=(ic == NC4 - 1))
    nc.vector.tensor_add(pv, ps3, Bf["bv"][0:POOL_PC, :])
    nc.sync.dma_start(g["contrib"][0, OFF_PV:AG_TOT].rearrange("(n c) -> n c", n=POOL_PC), pv)
    if "pk" in g["dbg"]:
        nc.sync.dma_start(g["dbg"]["pk"][:, :].rearrange("d n -> (d n)"), g["contrib"][0, OFF_PK:OFF_PV])
        nc.sync.dma_start(g["dbg"]["pv"][:, :].rearrange("n d -> (n d)"), g["contrib"][0, OFF_PV:AG_TOT])


def _proj_T(nc, ps_pool, w, bias_p, xt, dst, npix, piece):
    """dst[:, oc, pix] = (W.T @ x^T)[oc chunk] + bias (transposed proj)."""
    n_p = (npix + piece - 1) // piece
    for oc in range(NC4):
        for p in range(n_p):
            s = p * piece
            e = min(npix, s + piece)
            ps = ps_pool.tile([128, piece], dt.float32, tag="big")
            for ic in range(NC4):
                nc.tensor.matmul(ps[:, 0:e - s], w[:, ic, oc * 128:(oc + 1) * 128],
                                 xt[:, ic, s:e], start=(ic == 0), stop=(ic == NC4 - 1))
            nc.scalar.activation(dst[:, oc, s:e], ps[:, 0:e - s], AF.Identity,
                                 bias=bias_p[:, oc:oc + 1])


def _modulator(nc, tc, sb, ps_big, ps_mid, mw, mod_in, hs, tag):
    """Modulator on transposed f01 (k|v)+mask data [128, 4, 960] (10 rows
    incl halo). Returns modulated transposed [128, 4, 960] (rows 1..9 valid)."""
    HP = STRIP_H + 2
    PIX01 = 2 * HP * WI
    outT = sb.tile([128, NC4, 2, STRIP_H, WI], dt.float32, tag=f"modo{tag}")
    with tc.tile_pool(name=f"sbM{tag}", bufs=1) as sbm:
        # conv1 1x1 512->128 (+bias) + leaky relu -> padded rows [128,2,10,50]
        lx1 = sbm.tile([128, 2, HP, 50], dt.float32, tag="lx1")
        nc.vector.memset(lx1, 0.0)
        for f in range(2):
            s = f * (HP * WI)
            ps = ps_mid.tile([128, HP * WI], dt.float32, tag="mid")
            for ic in range(NC4):
                nc.tensor.matmul(ps, mw["sq"][:, ic, :],
                                 mod_in[:, ic, s:s + HP * WI],
                                 start=(ic == 0), stop=(ic == NC4 - 1))
            nc.scalar.activation(lx1[:, f, :, 1:49],
                                 ps.rearrange("p (y x) -> p y x", y=HP),
                                 AF.Identity, bias=mw["sqb"][:, 0:1])
            lint = lx1[:, f, :, 1:49]
            ltmp = sbm.tile([128, HP, 48], dt.float32, tag="ltmp", name="ltmp")
            nc.vector.tensor_scalar_mul(ltmp, lint, 0.2)
            nc.vector.tensor_max(lint, lint, ltmp)
        # zero the halo rows at image edges (conv zero-padding semantics)
        for f in range(2):
            nc.vector.tensor_scalar_mul(lx1[:, f, 0, :], lx1[:, f, 0, :],
                                        hs[:, 0:1])
            nc.vector.tensor_scalar_mul(lx1[:, f, HP - 1, :],
                                        lx1[:, f, HP - 1, :], hs[:, 1:2])
        # conv2 3x3 128->128 (+bias) + SiLU -> lx2 [128, 2, 384]
        lx2 = sbm.tile([128, 2, STRIP_H * WI], dt.float32, tag="lx2")
        for f in range(2):
            ps = ps_mid.tile([128, STRIP_H * WI], dt.float32, tag="mid")
            ti = 0
            for dy in (-1, 0, 1):
                for dx in (-1, 0, 1):
                    rhs = lx1[:, f, 1 + dy:1 + dy + STRIP_H, 1 + dx:1 + dx + WI]
                    nc.tensor.matmul(ps, mw["f"][:, ti, :], rhs,
                                     start=(ti == 0), stop=(ti == 8))
                    ti += 1
            sg = sbm.tile([128, STRIP_H * WI], dt.float32, tag="modsg")
            nc.scalar.activation(sg, ps, AF.Sigmoid, bias=mw["fb"][:, 0:1])
            tmp = sbm.tile([128, STRIP_H * WI], dt.float32, tag="modt")
            nc.scalar.activation(tmp, ps, AF.Identity, bias=mw["fb"][:, 0:1])
            nc.vector.tensor_mul(lx2[:, f, :], tmp, sg)
        # conv3 1x1 128->512 + bias -> transposed tile (strip rows 1..9)
        for f in range(2):
            for oc in range(NC4):
                ps = ps_mid.tile([128, STRIP_H * WI], dt.float32, tag="mid")
                nc.tensor.matmul(ps, mw["un"][:, oc * 128:(oc + 1) * 128],
                                 lx2[:, f, :], start=True, stop=True)
                nc.scalar.activation(outT[:, oc, f],
                                     ps.rearrange("p (y x) -> p y x", y=8),
                                     AF.Identity, bias=mw["unb"][:, oc:oc + 1])
    return outT


def _stage_b(nc, tc, g, sb0, ps_big, ps_mid, ps_y, W, Bp, Bf, qT, ident):
    ag = g["agout"]
    kgv = ag[:, OFF_K:OFF_V].rearrange("s (a p t y x) -> p s a t y x",
                                       a=NC4, p=128, t=T, y=STRIP_H)
    vgv = ag[:, OFF_V:OFF_PK].rearrange("s (t y x c) -> s t y x c",
                                        t=T, y=STRIP_H, x=64)
    pkgv = ag[:, OFF_PK:OFF_PV].rearrange("s (a p n) -> p s a n", a=NC4, p=128)
    pvgv = ag[:, OFF_PV:AG_TOT].rearrange("s (n c) -> s n c", n=POOL_PC)

    with (
        tc.tile_pool(name="sbB", bufs=2) as sb,
        tc.tile_pool(name="sbBig", bufs=1) as sbig,
        tc.tile_pool(name="sbP", bufs=1) as sbP,
        tc.tile_pool(name="dramp", bufs=4, space="DRAM") as dramp,
    ):
        # pooled keys resident: pk_sb [128, 4, 8, 72], pv_sb [72, 8, 512]
        pk_sb = sbP.tile([128, NC4, N_CORES, POOL_PC], dt.float32)
        for oc in range(NC4):
            nc.sync.dma_start(pk_sb[:, oc, :, :], pkgv[:, :, oc, :])
        pv_sb = sbP.tile([POOL_PC, N_CORES, DIM], dt.float32)
        for s in range(N_CORES):
            nc.sync.dma_start(pv_sb[:, s, :], pvgv[s])

        # dynamic-offset registers (per window: prev,cur,next,x0,x_own)
        meta_t = sbP.tile([1, META_LEN], dt.int32)
        nc.sync.dma_start(meta_t, g["meta"][:].unsqueeze(0))
        regs = []
        for i in range(WPC * 5):
            r = nc.alloc_register(mybir.EngineType.SP, f"mreg{i}")
            nc.sync.reg_load(r, meta_t[0:1, i:i + 1])
            regs.append(nc.sync.snap(r))

        for wi in range(WPC):
            r_prev, r_cur, r_next, r_x0, r_xo = regs[wi * 5:wi * 5 + 5]
            ds = bass.ds
            # ---- k patch [128, 4(oc), 4(piece), 4(f), 64(y4 x16)] ----
            kp = sbig.tile([128, NC4, 4, T, 64], dt.float32, tag="kp")
            pieces = [(r_prev, 4), (r_cur, 0), (r_cur, 4), (r_next, 0)]
            for oc in range(NC4):
                for pi, (slot, y0) in enumerate(pieces):
                    nc.sync.dma_start(
                        kp[:, oc, pi].rearrange("p t (y x) -> p t y x", y=4),
                        kgv[:, ds(slot, 1), oc, :, y0:y0 + 4, ds(r_x0, 16)]
                           .squeeze(1))
            # ---- v patch [128, 8, 512]: 16 (piece,f) blocks of 64 pix ----
            vp = sbig.tile([128, 8, DIM], dt.float32, tag="vp")
            for pi, (slot, y0) in enumerate(pieces):
                for f in range(T):
                    b = pi * 4 + f
                    nc.sync.dma_start(
                        vp[64 * (b % 2):64 * (b % 2) + 64, b // 2, :],
                        vgv[ds(slot, 1), f, y0:y0 + 4, ds(r_x0, 16), :]
                           .squeeze(0))
            # ---- v own-window [64, 4, 512] ----
            vown = sbig.tile([64, T, DIM], dt.float32, tag="vown")
            for f in range(T):
                nc.sync.dma_start(
                    vown[:, f, :],
                    vgv[ds(r_cur, 1), f, :, ds(r_xo, 8), :]
                       .squeeze(0))
            # ---- k^T own-window [128, 4(oc), 4(f), 64] ----
            kown = sbig.tile([128, NC4, T, WN], dt.float32, tag="kown")
            for oc in range(NC4):
                nc.sync.dma_start(
                    kown[:, oc].rearrange("p f (y x) -> p f y x", y=WH),
                    kgv[:, ds(r_cur, 1), oc, :, :, ds(r_xo, 8)].squeeze(1))
            # ---- flag bcast [128,1] ----
            flb = sb.tile([128, 1], dt.float32, tag="flb")
            nc.sync.dma_start(flb, g["flags_d"][wi:wi + 1].unsqueeze(0)
                              .to_broadcast([128, 1]))

            q_w = qT[:, :, wi * NQ:(wi + 1) * NQ]  # [128, 4, 256]
            kpf = kp.rearrange("p a x t n -> p a (x t n)")  # [128,4,1024]
            yfin = sbig.tile([128, N_HEAD, NQ], dt.float32, tag="yfin")

            for h in range(N_HEAD):
                # ================= global attention =================
                pT = sbig.tile([128, NCHUNK, NQ], dt.float32, tag="pT")
                for j in range(NCHUNK):
                    n = KCH[j]
                    ps = ps_mid.tile([128, NQ], dt.float32, tag="mid")
                    if j < 8:
                        lhs = kpf[:, h, j * 128:(j + 1) * 128]
                    else:
                        lhs = pk_sb[:, h, j - 8, :]
                    nc.tensor.matmul(ps[0:n, :], lhs, q_w[:, h, :],
                                     start=True, stop=True)
                    nc.scalar.activation(pT[0:n, j, :], ps[0:n, :], AF.Exp,
                                         scale=SCALE)
                # denominators: p_acc -> transpose -> row sums -> recip row
                p_acc = sb.tile([128, NQ], dt.float32, tag="pacc")
                nc.vector.tensor_add(p_acc, pT[:, 0, :], pT[:, 1, :])
                for j in range(2, 8):
                    nc.vector.tensor_add(p_acc, p_acc, pT[:, j, :])
                for j in range(8, NCHUNK):
                    nc.vector.tensor_add(p_acc[0:POOL_PC, :], p_acc[0:POOL_PC, :],
                                         pT[0:POOL_PC, j, :])
                rrow_g = _recip_row(nc, sb, ps_mid, p_acc, ident, "g", dram=dramp)
                # AV accumulate
                psy = ps_y.tile([128, NQ], dt.float32, tag="y")
                for j in range(NCHUNK):
                    n = KCH[j]
                    if j < 8:
                        lhs = vp[0:n, j, h * 128:(h + 1) * 128]
                    else:
                        lhs = pv_sb[:, j - 8, h * 128:(h + 1) * 128]
                    nc.tensor.matmul(psy, lhs, pT[0:n, j, :],
                                     start=(j == 0), stop=(j == NCHUNK - 1))
                rgB = sb.tile([128, NQ], dt.float32, tag="rgB", name="rgB")
                nc.gpsimd.dma_start(rgB, rrow_g[:, :].to_broadcast([128, NQ]))
                y_g = sb.tile([128, NQ], dt.float32, tag="yg_sb")
                nc.vector.tensor_mul(y_g, psy, rgB)

                # ================= local attention =================
                psl = ps_mid.tile([64, NQ], dt.float32, tag="mid")
                for f in range(T):
                    nc.tensor.matmul(psl[:, f * WN:(f + 1) * WN],
                                     kown[:, h, f, :],
                                     q_w[:, h, f * WN:(f + 1) * WN],
                                     start=True, stop=True)
                ploc = sb.tile([64, NQ], dt.float32, tag="ploc")
                nc.scalar.activation(ploc, psl, AF.Exp, scale=SCALE)
                rrow_l = _recip_row(nc, sb, ps_mid, ploc, ident, "l", dram=dramp, parts=64)
                psyl = ps_y.tile([128, NQ], dt.float32, tag="y")
                for f in range(T):
                    # own-window keys for frame f in ploc rows: order (piece,y,x8)
                    nc.tensor.matmul(psyl[:, f * WN:(f + 1) * WN],
                                     vown[:, f, h * 128:(h + 1) * 128],
                                     ploc[:, f * WN:(f + 1) * WN],
                                     start=True, stop=True)
                rlB = sb.tile([128, NQ], dt.float32, tag="rlB", name="rlB")
                nc.gpsimd.dma_start(rlB, rrow_l[:, :].to_broadcast([128, NQ]))
                y_l = sb.tile([128, NQ], dt.float32, tag="yl_sb")
                nc.vector.tensor_mul(y_l, psyl, rlB)

                # ---- blend: y = y_l + flag*(y_g - y_l) ----
                dlt = sb.tile([128, NQ], dt.float32, tag="dlt")
                nc.vector.tensor_sub(dlt, y_g, y_l)
                nc.vector.tensor_scalar_mul(dlt, dlt, flb[:, 0:1])
                nc.vector.tensor_add(yfin[:, h, :], y_l, dlt)

            # ================= output projection =================
            for fp in range(2):
                pso = ps_big.tile([128, DIM], dt.float32, tag="big")
                for h in range(N_HEAD):
                    nc.tensor.matmul(pso, yfin[:, h, fp * 128:(fp + 1) * 128],
                                     W["wp"][:, h, :],
                                     start=(h == 0), stop=(h == N_HEAD - 1))
                osb = sb.tile([128, DIM], dt.float32, tag="osb")
                nc.vector.tensor_add(osb, pso, Bf["bp"])
                nc.sync.dma_start(
                    g["out_win"][wi, 2 * fp:2 * fp + 2, :, :]
                        .rearrange("t p c -> (t p) c"), osb)


def _recip_row(nc, sb, ps_mid, p_acc, ident, tag, dram=None, parts=128):
    """sum over partitions of p_acc[parts, 256] -> reciprocal -> [1,256] row."""
    sums = sb.tile([128, 2], dt.float32, tag=f"sum{tag}")
    for half in range(2):
        ps = ps_mid.tile([128, 128], dt.float32, tag="mid")
        nc.tensor.transpose(ps[:, 0:parts],
                            p_acc[0:parts, half * 128:(half + 1) * 128],
                            ident[0:parts, 0:parts])
        nc.vector.reduce_sum(sums[:, half:half + 1], ps[:, 0:parts], axis=AX.X)
    rec = sb.tile([128, 2], dt.float32, tag=f"rec{tag}")
    nc.vector.reciprocal(rec, sums)
    rd = dram.tile([1, NQ], dt.float32, tag="rrow_d", name="rrow_d")
    nc.sync.dma_start(rd[0:1, 0:128], rec[:, 0:1])
    nc.sync.dma_start(rd[0:1, 128:256], rec[:, 1:2])
    return rd


# ==================== host side ====================

def _host_inputs(inputs, debug=False):
    x = np.asarray(inputs["x"], np.float32)[0]  # [4,48,48,512]
    mask = np.asarray(inputs["mask"], np.float32)[0, :, :, :, 0]  # [2,48,48]

    pool_ind = np.zeros((2 * WI, PGRID), np.float32)
    for y in range(2):
        for xx in range(WI):
            pool_ind[y * WI + xx, xx // PW] = 1.0 / (PH * PW)

    common = dict(
        wq=np.asarray(inputs["Wq"], np.float32), bq=np.asarray(inputs["bq"], np.float32),
        wk=np.asarray(inputs["Wk"], np.float32), bk=np.asarray(inputs["bk"], np.float32),
        wv=np.asarray(inputs["Wv"], np.float32), bv=np.asarray(inputs["bv"], np.float32),
        wp=np.asarray(inputs["Wp"], np.float32), bp=np.asarray(inputs["bp"], np.float32),
        pool_b=np.asarray(inputs["pool_b"], np.float32),
        pool_ind=pool_ind,
    )
    for tag, pre in (("k", "kmod"), ("v", "vmod")):
        common[f"{tag}sq"] = np.ascontiguousarray(
            np.asarray(inputs[f"{pre}_sq_w"], np.float32)[:, :, 0, 0].T)
        common[f"{tag}sqb"] = np.asarray(inputs[f"{pre}_sq_b"], np.float32)
        fw = np.asarray(inputs[f"{pre}_f_w"], np.float32)
        common[f"{tag}f"] = np.ascontiguousarray(
            np.stack([fw[:, :, dy, dx].T for dy in range(3) for dx in range(3)]))
        common[f"{tag}fb"] = np.asarray(inputs[f"{pre}_f_b"], np.float32)
        common[f"{tag}un"] = np.ascontiguousarray(
            np.asarray(inputs[f"{pre}_un_w"], np.float32)[:, :, 0, 0].T)
        common[f"{tag}unb"] = np.asarray(inputs[f"{pre}_un_b"], np.float32)

    in_maps = []
    for c in range(N_CORES):
        m = dict(common)
        # strip rows with halo
        if c < N_STRIP:
            r0 = c * STRIP_H
            xs = np.zeros((T, STRIP_H + 2, WI, DIM), np.float32)
            ms = np.zeros((L_T, STRIP_H + 2, WI), np.float32)
            lo, hi = max(0, r0 - 1), min(HI, r0 + STRIP_H + 1)
            xs[:, lo - (r0 - 1):lo - (r0 - 1) + hi - lo] = x[:, lo:hi]
            ms[:, lo - (r0 - 1):lo - (r0 - 1) + hi - lo] = mask[:, lo:hi]
            m["x_strip"] = xs
            m["mask_strip"] = ms
            m["halo_scale"] = np.array(
                [0.0 if r0 == 0 else 1.0,
                 0.0 if r0 + STRIP_H == HI else 1.0], np.float32)
        else:
            m["x_strip"] = np.zeros((T, STRIP_H + 2, WI, DIM), np.float32)
            m["mask_strip"] = np.zeros((L_T, STRIP_H + 2, WI), np.float32)
            m["halo_scale"] = np.ones(2, np.float32)
        # window inputs
        xw = np.zeros((WPC, T, WN, DIM), np.float32)
        mw = np.zeros((L_T, WPC, WN), np.float32)
        for k, w in enumerate(WIN_ASSIGN[c]):
            i, j = w // N_WW, w % N_WW
            blk = x[:, 8 * i:8 * i + 8, 8 * j:8 * j + 8, :]
            xw[k] = blk.reshape(T, WN, DIM)
            mw[:, k] = mask[:, 8 * i:8 * i + 8, 8 * j:8 * j + 8].reshape(L_T, WN)
        m["x_win"] = xw
        m["mask_win"] = mw
        # pool rows: 72 pooled cells = frame c//2, cell-rows 6*(c%2)..+6
        f, pr0 = c // 2, 6 * (c % 2)
        m["x_pool"] = np.ascontiguousarray(x[f, pr0 * 4:pr0 * 4 + 24])
        m["meta"] = _meta_for_core(c)
        in_maps.append(m)
    return in_maps


def _get_nc(debug=False):
    key = bool(debug)
    if key not in _NC_CACHE:
        _NC_CACHE[key] = build_nc(debug=debug)
    return _NC_CACHE[key]


def run_spmd(inputs, debug=False):
    nc = _get_nc(debug=debug)
    in_maps = _host_inputs(inputs, debug=debug)
    res = run_bass_kernel_spmd(nc, in_maps, list(range(N_CORES)))
    return res


def assemble(results):
    out = np.zeros((T, HI, WI, DIM), np.float32)
    done = set()
    for c in range(N_CORES):
        ow = results[c]["out_win"]  # [5,4,64,512]
        for k, w in enumerate(WIN_ASSIGN[c]):
            if w in done:
                continue
            done.add(w)
            i, j = w // N_WW, w % N_WW
            out[:, 8 * i:8 * i + 8, 8 * j:8 * j + 8, :] = \
                ow[k].reshape(T, WH, WW, DIM)
    return out[None]


def kernel(**inputs):
    res = run_spmd(inputs)
    return assemble(res.results)


_CALLABLE_CACHE = {}


def _get_callable(debug=False):
    """Build the sharded jitted callable once (mirrors run_bass_via_pjrt)."""
    key = bool(debug)
    if key in _CALLABLE_CACHE:
        return _CALLABLE_CACHE[key]
    import jax
    from jax.sharding import Mesh, PartitionSpec
    from jax.experimental.shard_map import shard_map
    from concourse import bass2jax, mybir as _mb

    nc = _get_nc(debug=debug)
    bass2jax.install_neuronx_cc_hook()
    in_names, out_names, out_avals, zero_outs = [], [], [], []
    pname = nc.partition_id_tensor.name if nc.partition_id_tensor else None
    for alloc in nc.m.functions[0].allocations:
        if not isinstance(alloc, _mb.MemoryLocationSet):
            continue
        name = alloc.memorylocations[0].name
        if alloc.kind == "ExternalInput":
            if name != pname:
                in_names.append(name)
        elif alloc.kind == "ExternalOutput":
            out_names.append(name)
            shape = tuple(alloc.tensor_shape)
            dtp = _mb.dt.np(alloc.dtype)
            out_avals.append(jax.core.ShapedArray(shape, dtp))
            zero_outs.append(np.zeros(shape, dtp))
    n_params = len(in_names)
    all_in = list(in_names) + list(out_names)
    if pname is not None:
        all_in.append(pname)

    def _body(*args):
        ops = list(args)
        if pname is not None:
            ops.append(bass2jax.partition_id_tensor())
        return tuple(bass2jax._bass_exec_p.bind(
            *ops, out_avals=tuple(out_avals), in_names=tuple(all_in),
            out_names=tuple(out_names), lowering_input_output_aliases=(),
            sim_require_finite=True, sim_require_nnan=True, nc=nc))

    devices = jax.devices()[:N_CORES]
    mesh = Mesh(np.asarray(devices), ("core",))
    n_outs = len(out_names)
    sharded = jax.jit(
        shard_map(_body, mesh=mesh,
                  in_specs=(PartitionSpec("core"),) * (n_params + n_outs),
                  out_specs=(PartitionSpec("core"),) * n_outs,
                  check_rep=False),
        donate_argnums=tuple(range(n_params, n_params + n_outs)),
        keep_unused=True)
    info = (sharded, in_names, out_names, out_avals, zero_outs)
    _CALLABLE_CACHE[key] = info
    return info


def timed_run(inputs, iters=4, debug=False):
    """Run via a cached jitted callable; returns (results, best_wall_s)."""
    import time as _time
    import jax
    sharded, in_names, out_names, out_avals, zero_outs = _get_callable(debug)
    in_maps = _host_inputs(inputs, debug=debug)
    dbgz = np.zeros((1, 2), np.uint32)  # dbg_addr placeholder (debug builds)
    concat_in = [np.concatenate(
        [np.asarray(in_maps[c].get(n, dbgz)) for c in range(N_CORES)], 0)
        for n in in_names]
    concat_in = [jax.device_put(a) for a in concat_in]
    best = None
    out_arrs = None
    for _ in range(iters):
        zeros = [np.zeros((N_CORES * z.shape[0],) + z.shape[1:], z.dtype)
                 for z in zero_outs]
        t0 = _time.perf_counter()
        out_arrs = sharded(*concat_in, *zeros)
        jax.block_until_ready(out_arrs)
        dt_ = _time.perf_counter() - t0
        best = dt_ if best is None else min(best, dt_)
    results = [
        {n: np.asarray(out_arrs[i]).reshape(N_CORES, *out_avals[i].shape)[c]
         for i, n in enumerate(out_names)}
        for c in range(N_CORES)
    ]
    return results, best



# revision 21
# speedup vs baseline: 1.8080x; 1.8080x over previous
"""Trainium2 Bass kernel for nn_ErrorAwareSelfAttention (8 NeuronCores).

Design (bf16 rev):
- Stage A sharded by 8-image-row strips on cores 0-5 (window-aligned):
  k^T (ch-major) + v (pixel-major) projections, Modulator convs on frames
  0,1 (1-px halo from host-padded x_strip), pooled tokens sharded over all
  8 cores. One AllGather publishes k^T / v / pk^T / pv (all bf16).
- Stage B sharded 5-padded-windows per core: global attention per
  (window, head) with keys = 16x16 halo patch (1024) + 576 pooled keys in
  13 partition-packed chunks (8 patch + 4x128+64 pooled); local per-frame
  attention over the own 8x8 window (keys sliced out of the patch tile);
  blend by the per-window mask flag; output projection; host scatters.
- All matmuls bf16 (1 cyc/row vs 4 for fp32); PSUM accumulation fp32.
  x^T via DMA-transpose (xbar) instead of PE transposes.
- Softmax: scores stay transposed (keys on partitions, 256 queries free).
  exp batched in groups of 4 chunks; denominators via ones-vector matmul
  accumulated in PSUM ([33,256] tile rows 0=global / 32=local), recip on
  DVE, partition-broadcast of the reciprocal row via PE outer product.
  No DRAM round trip, no gpsimd broadcast.
- The 13th (64-key pool remainder) chunk and the 4x64 local scores share
  one [128,256] PSUM tile (rows 0:64 / 64:128) and a single exp.
"""

import math
import sys

sys.path.insert(0, "/opt/trn_rl_repo")

import numpy as np
import ml_dtypes

import concourse.bass as bass
import concourse.mybir as mybir
import concourse.tile as tile
from concourse import bacc
from concourse.bass_utils import run_bass_kernel_spmd
from concourse.masks import make_identity

dt = mybir.dt
BF = dt.bfloat16
AF = mybir.ActivationFunctionType
AX = mybir.AxisListType
NPBF = ml_dtypes.bfloat16

# ---------------- problem constants (hardcoded) ----------------
DIM = 512
N_HEAD = 4
CH = 128
WH = WW = 8
EH = EW = 4
PH = PW = 4
B, T, HI, WI = 1, 4, 48, 48
L_T = 2
N_WH = N_WW = 6
NW = 36
WN = 64
SCALE = 1.0 / math.sqrt(CH)
N_CORES = 8
NC4 = 4  # 512 / 128 channel chunks
GSZ = 2  # score chunks per exp group (PSUM bank sized)

# stage A strips: 6 strips of 8 image rows (cores 0-5). x_strip has 1-row halo.
STRIP_H = 8
N_STRIP = 6
# stage B: 5 windows per core (padded; 36 windows total)
WPC = 5
_bounds = [int(NW * c / N_CORES) for c in range(N_CORES + 1)]
WIN_ASSIGN = []  # per core: list of 5 window ids (last repeated as padding)
for c in range(N_CORES):
    ws = list(range(_bounds[c], _bounds[c + 1]))
    while len(ws) < WPC:
        ws.append(ws[-1])
    WIN_ASSIGN.append(ws)

# pooled tokens: 12x12 per frame -> 576 rows, 72 per core
PGRID = HI // PH  # 12
NPOOL = T * PGRID * PGRID  # 576
POOL_PC = NPOOL // N_CORES  # 72

PATCH_PIX = 4 * 16 * 16  # 1024 keys/window from the halo patch (4 frames)
NKEYS = PATCH_PIX + NPOOL  # 1600 global keys
NQ = T * WN  # 256 queries per window
# 13 key chunks: 8 patch(128) + pooled 128,128,128,128,64
KCH = [128] * 12 + [64]
NCHUNK = 13
NGRP = 6  # exp groups of GSZ chunks over chunks 0..11; chunk 12 rides w/ local
# flat AllGather packing offsets (bf16 elements)
OFF_K = 0
OFF_V = OFF_K + DIM * T * STRIP_H * 64
OFF_PK = OFF_V + T * STRIP_H * 64 * DIM
OFF_PV = OFF_PK + DIM * POOL_PC
AG_TOT = OFF_PV + POOL_PC * DIM

_NC_CACHE = {}


def _meta_for_core(c):
    """Per-window dynamic DMA registers: slot_prev, slot_cur, slot_next,
    x0 (patch x start in the 64-wide margin layout), x_own (=8j)."""
    vals = []
    for w in WIN_ASSIGN[c]:
        i, j = w // N_WW, w % N_WW
        vals += [(i - 1) % N_STRIP, i, (i + 1) % N_STRIP, (8 * j - 4) % 48, 8 * j]
    vals += [0] * (32 - len(vals) % 32 if len(vals) % 32 else 0)
    return np.asarray(vals[: ((len(vals) + 31) // 32) * 32], np.int32)


META_LEN = len(_meta_for_core(0))


def build_nc(debug=False):
    nc = bacc.Bacc("TRN2", target_bir_lowering=False, debug=True)

    # ---------------- I/O ----------------
    x_strip = nc.dram_tensor("x_strip", [T, STRIP_H + 2, WI, DIM], BF,
                             kind="ExternalInput")
    x_win = nc.dram_tensor("x_win", [WPC, T, WN, DIM], BF,
                           kind="ExternalInput")
    x_pool = nc.dram_tensor("x_pool", [POOL_PC // PGRID * PH, WI, DIM],
                            BF, kind="ExternalInput")  # [24,48,512]
    mask_strip = nc.dram_tensor("mask_strip", [L_T, STRIP_H + 2, WI],
                                BF, kind="ExternalInput")
    mask_win = nc.dram_tensor("mask_win", [L_T, WPC, WN], dt.float32,
                              kind="ExternalInput")
    halo_scale = nc.dram_tensor("halo_scale", [2], dt.float32,
                                kind="ExternalInput")
    meta = nc.dram_tensor("meta", [META_LEN], dt.int32, kind="ExternalInput")
    pool_ind = nc.dram_tensor("pool_ind", [2 * WI, PGRID], BF,
                              kind="ExternalInput")  # [96,12] avg indicator

    wq_d = nc.dram_tensor("wq", [DIM, DIM], BF, kind="ExternalInput")
    wk_d = nc.dram_tensor("wk", [DIM, DIM], BF, kind="ExternalInput")
    wv_d = nc.dram_tensor("wv", [DIM, DIM], BF, kind="ExternalInput")
    wp_d = nc.dram_tensor("wp", [DIM, DIM], BF, kind="ExternalInput")
    bq_d = nc.dram_tensor("bq", [DIM], dt.float32, kind="ExternalInput")
    bk_d = nc.dram_tensor("bk", [DIM], dt.float32, kind="ExternalInput")
    bv_d = nc.dram_tensor("bv", [DIM], dt.float32, kind="ExternalInput")
    bp_d = nc.dram_tensor("bp", [DIM], dt.float32, kind="ExternalInput")
    pool_b_d = nc.dram_tensor("pool_b", [DIM], dt.float32, kind="ExternalInput")
    # modulator weights, host pre-transposed to [in,out]
    mods = {}
    for tag in ("k", "v"):
        mods[tag] = dict(
            sq=nc.dram_tensor(f"{tag}sq", [DIM, 128], BF, kind="ExternalInput"),
            sqb=nc.dram_tensor(f"{tag}sqb", [128], dt.float32, kind="ExternalInput"),
            f=nc.dram_tensor(f"{tag}f", [9, 128, 128], BF, kind="ExternalInput"),
            fb=nc.dram_tensor(f"{tag}fb", [128], dt.float32, kind="ExternalInput"),
            un=nc.dram_tensor(f"{tag}un", [128, DIM], BF, kind="ExternalInput"),
            unb=nc.dram_tensor(f"{tag}unb", [DIM], dt.float32, kind="ExternalInput"),
        )

    out_win = nc.dram_tensor("out_win", [WPC, T, WN, DIM], dt.float32,
                             kind="ExternalOutput")
    dbg = {}
    if debug:
        dbg["k_contrib"] = nc.dram_tensor("dbg_k", [DIM, T, STRIP_H, 64],
                                          BF, kind="ExternalOutput")
        dbg["v_contrib"] = nc.dram_tensor("dbg_v", [T, STRIP_H, 64, DIM],
                                          BF, kind="ExternalOutput")
        dbg["pk"] = nc.dram_tensor("dbg_pk", [DIM, POOL_PC], BF,
                                   kind="ExternalOutput")
        dbg["pv"] = nc.dram_tensor("dbg_pv", [POOL_PC, DIM], BF,
                                   kind="ExternalOutput")
        dbg["q"] = nc.dram_tensor("dbg_q", [NC4, 128, WPC * NQ], BF,
                                  kind="ExternalOutput")
        dbg["flags"] = nc.dram_tensor("dbg_flags", [WPC], dt.float32,
                                      kind="ExternalOutput")

    # internal DRAM for collective (single flat packed buffer)
    contrib = nc.dram_tensor("contrib", [1, AG_TOT], BF)
    agout = nc.dram_tensor("agout", [N_CORES, AG_TOT], BF,
                           addr_space="Shared")
    flags_d = nc.dram_tensor("flags_d", [WPC], dt.float32)

    with tile.TileContext(nc, num_cores=N_CORES) as tc:
        _program(nc, tc, locals())
    nc.compile()
    return nc


def _program(nc, tc, g):
    with (
        tc.tile_pool(name="consts", bufs=1) as consts,
        tc.tile_pool(name="wpool", bufs=1) as wpool,
    ):
        ident = consts.tile([128, 128], BF)
        make_identity(nc, ident)
        ones_col = consts.tile([128, 1], BF)
        nc.vector.memset(ones_col, 1.0)
        ones_row = consts.tile([33, 128], BF)
        nc.vector.memset(ones_row, 1.0)

        # weights as [128 (in chunk), 4 in-chunks, 512 out]
        W = {}
        for nm in ("wq", "wk", "wv", "wp"):
            t = wpool.tile([128, NC4, DIM], BF, tag=nm)
            nc.sync.dma_start(t, g[nm + "_d"][:, :].rearrange("(a p) o -> p a o", p=128))
            W[nm] = t
        # per-partition bias tiles [128,1] x4 chunks: store as [128, 4]
        Bp = {}
        for nm in ("bq", "bk", "bv"):
            t = wpool.tile([128, NC4], dt.float32, tag=nm + "p")
            nc.sync.dma_start(t, g[nm + "_d"][:].rearrange("(a p) -> p a", p=128))
            Bp[nm] = t
        # free-axis broadcast bias tiles [128, 512]
        Bf = {}
        for nm in ("bv", "bp", "pool_b"):
            t = wpool.tile([128, DIM], dt.float32, tag=nm + "f")
            nc.sync.dma_start(t, g[nm + "_d"][:].unsqueeze(0).to_broadcast([128, DIM]))
            Bf[nm] = t
        MW = {}
        for tag in ("k", "v"):
            m = g["mods"][tag]
            MW[tag] = dict(
                sq=wpool.tile([128, NC4, 128], BF, tag=f"{tag}sqw", name=f"{tag}sqw"),
                sqb=wpool.tile([128, 1], dt.float32, tag=f"{tag}sqbw", name=f"{tag}sqbw"),
                f=wpool.tile([128, 9, 128], BF, tag=f"{tag}fw", name=f"{tag}fw"),
                fb=wpool.tile([128, 1], dt.float32, tag=f"{tag}fbw", name=f"{tag}fbw"),
                un=wpool.tile([128, DIM], BF, tag=f"{tag}unw", name=f"{tag}unw"),
                unb=wpool.tile([128, NC4], dt.float32, tag=f"{tag}unbw", name=f"{tag}unbw"),
            )
            nc.sync.dma_start(MW[tag]["sq"], m["sq"][:, :].rearrange("(a p) o -> p a o", p=128))
            nc.sync.dma_start(MW[tag]["sqb"], m["sqb"][:].unsqueeze(1))
            nc.sync.dma_start(MW[tag]["f"], m["f"][:, :, :].rearrange("n p o -> p n o"))
            nc.sync.dma_start(MW[tag]["fb"], m["fb"][:].unsqueeze(1))
            nc.sync.dma_start(MW[tag]["un"], m["un"][:, :])
            nc.sync.dma_start(MW[tag]["unb"], m["unb"][:].rearrange("(a p) -> p a", p=128))
        pool_ind_t = wpool.tile([2 * WI, PGRID], BF)
        nc.sync.dma_start(pool_ind_t, g["pool_ind"][:, :])
        hs = wpool.tile([128, 2], dt.float32)
        nc.sync.dma_start(hs, g["halo_scale"][:].unsqueeze(0).to_broadcast([128, 2]))

        with tc.tile_pool(name="sbB0", bufs=1) as sbB0:
            qT = sbB0.tile([128, NC4, WPC * NQ], BF)
            # ============ stage A (+ qT proj + flags), own PSUM pools ======
            with (
                tc.tile_pool(name="ps_big", bufs=2, space="PSUM") as ps_big,
                tc.tile_pool(name="ps_mid", bufs=4, space="PSUM") as ps_mid,
                tc.tile_pool(name="ps_bf", bufs=2, space="PSUM") as ps_bf,
            ):
                with tc.tile_pool(name="sbA", bufs=1) as sbA:
                    _stage_a(nc, tc, g, sbA, ps_big, ps_mid, ps_bf, W, Bp, Bf,
                             MW, pool_ind_t, hs, ident)

                # ---- stage B prologue: q^T (no AG dep; overlaps AG) ----
                with tc.tile_pool(name="sbXW", bufs=1) as sbXW:
                    xtw = sbXW.tile([128, NC4, WPC * NQ], BF)
                    xw_flat = g["x_win"][:, :, :, :].rearrange("w t p c -> (w t p) c")
                    for ic in range(NC4):
                        nc.sync.dma_start(xtw[:, ic, :],
                                          xw_flat[:, ic * 128:(ic + 1) * 128],
                                          transpose=True)
                    _proj_T(nc, ps_mid, W["wq"], Bp["bq"], xtw, qT,
                            WPC * NQ, 320, tag="mid")
                if "q" in g["dbg"]:
                    for oc in range(NC4):
                        nc.sync.dma_start(g["dbg"]["q"][oc, :, :], qT[:, oc, :])

                # ---------------- flags ----------------
                mwt = sbB0.tile([L_T, WPC * WN], dt.float32)
                nc.sync.dma_start(mwt, g["mask_win"][:, :, :].rearrange("l w p -> l (w p)"))
                mx = sbB0.tile([L_T, WPC, 1], dt.float32)
                nc.vector.reduce_max(mx, mwt.rearrange("l (w p) -> l w p", w=WPC),
                                     axis=AX.X, opt_input=False, opt_output=False)
                with tc.tile_pool(name="flg_d", bufs=1, space="DRAM") as flgp:
                    mx_d = flgp.tile([L_T, WPC], dt.float32)
                    nc.sync.dma_start(mx_d, mx[:, :, 0])
                    mrow = sbB0.tile([1, L_T * WPC], dt.float32)
                    nc.sync.dma_start(mrow, mx_d[:, :].rearrange("l w -> (l w)")
                                      .unsqueeze(0))
                msum = sbB0.tile([1, WPC], dt.float32)
                nc.vector.tensor_add(msum, mrow[:, 0:WPC], mrow[:, WPC:2 * WPC])
                fl = sbB0.tile([1, WPC], dt.float32)
                nc.scalar.activation(fl, msum, AF.Sign)
                nc.sync.dma_start(g["flags_d"][:].unsqueeze(0), fl[0:1, :])
                if "flags" in g["dbg"]:
                    nc.sync.dma_start(g["dbg"]["flags"][:].unsqueeze(0), fl[0:1, :])

            # ---------------- AllGather ----------------
            nc.gpsimd.collective_compute(
                "AllGather", mybir.AluOpType.bypass,
                ins=[g["contrib"][:, :]],
                outs=[g["agout"][:, :]],
                replica_groups=[list(range(N_CORES))],
            )

            # ================= stage B =================
            with (
                tc.tile_pool(name="ps_sc", bufs=2, space="PSUM") as ps_sc,
                tc.tile_pool(name="ps_cmb", bufs=2, space="PSUM") as ps_cmb,
                tc.tile_pool(name="ps_dn", bufs=2, space="PSUM") as ps_dn,
                tc.tile_pool(name="ps_y", bufs=2, space="PSUM") as ps_y,
            ):
                _stage_b(nc, tc, g, sbB0, ps_sc, ps_cmb, ps_dn, ps_y,
                         W, Bf, qT, ones_col, ones_row)


def _stage_a(nc, tc, g, sb, ps_big, ps_mid, ps_bf, W, Bp, Bf, MW,
             pool_ind_t, hs, ident):
    HP = STRIP_H + 2  # 10 rows incl halo
    PIX01 = 2 * HP * WI  # 960 f01 pixels (with halo rows)
    PIX23 = 2 * STRIP_H * WI  # 768 (no halo needed)

    xs = g["x_strip"]
    maskb = sb.tile([128, PIX01], BF)
    nc.sync.dma_start(maskb, g["mask_strip"][:, :, :].rearrange("l y x -> (l y x)")
                      .unsqueeze(0).to_broadcast([128, PIX01]))

    kT23 = sb.tile([128, NC4, PIX23], BF)
    kmod_in = sb.tile([128, NC4, PIX01], BF)  # k01+mask (mod input)
    vmod_in = sb.tile([128, NC4, PIX01], BF)
    vc = g["contrib"][0, OFF_V:OFF_PK].rearrange("(t y x c) -> t y x c",
                                                 t=T, y=STRIP_H, x=64)

    def _v_out(vt, fr, t):
        f, y = divmod(t, 4)  # (frame offset, 2-row group)
        nc.sync.dma_start(vc[fr + f, 2 * y:2 * y + 2, 0:48, :], vt)
        nc.sync.dma_start(vc[fr + f, 2 * y, 48:64, :], vt[0:16, :])
        nc.sync.dma_start(vc[fr + f, 2 * y + 1, 48:64, :], vt[48:64, :])

    with tc.tile_pool(name="sbXT", bufs=1) as sbXT:
        # x^T via DMA transpose: f01 all 10 rows, f23 middle 8 rows
        xt01 = sbXT.tile([128, NC4, PIX01], BF, tag="xt01")
        x01_flat = xs[0:2].rearrange("t y x c -> (t y x) c")
        for ic in range(NC4):
            nc.sync.dma_start(xt01[:, ic, :],
                              x01_flat[:, ic * 128:(ic + 1) * 128],
                              transpose=True)
        xt23 = sbXT.tile([128, NC4, PIX23], BF, tag="xt23")
        for f in range(2):
            xf_flat = xs[2 + f, 1:1 + STRIP_H].rearrange("y x c -> (y x) c")
            for ic in range(NC4):
                nc.sync.dma_start(
                    xt23[:, ic, f * 384:(f + 1) * 384],
                    xf_flat[:, ic * 128:(ic + 1) * 128],
                    transpose=True)
        # k/v f01 projections straight into modulator-input tiles (+mask)
        _proj_T(nc, ps_big, W["wk"], Bp["bk"], xt01, kmod_in, PIX01, 480)
        _proj_T(nc, ps_big, W["wv"], Bp["bv"], xt01, vmod_in, PIX01, 480)
        for ic in range(NC4):
            nc.vector.tensor_add(kmod_in[:, ic, :], kmod_in[:, ic, :], maskb)
            nc.vector.tensor_add(vmod_in[:, ic, :], vmod_in[:, ic, :], maskb)

        _proj_T(nc, ps_big, W["wk"], Bp["bk"], xt23, kT23, PIX23, 384)
        for t in range(8):
            f, grp = t // 4, t % 4
            s = f * 384 + grp * 96
            ps = ps_big.tile([96, DIM], dt.float32, tag="big")
            for ic in range(NC4):
                nc.tensor.matmul(ps, xt23[:, ic, s:s + 96],
                                 W["wv"][:, ic, :], start=(ic == 0),
                                 stop=(ic == NC4 - 1))
            vst = sb.tile([96, DIM], BF, tag="vst", name="vst")
            nc.vector.tensor_add(vst, ps, Bf["bv"][0:96, :])
            _v_out(vst, 2, t)

    # ---- modulators (replace f01) ----
    kT01m = _modulator(nc, tc, sb, ps_big, ps_mid, MW["k"], kmod_in, hs, "k")
    vT01m = _modulator(nc, tc, sb, ps_big, ps_mid, MW["v"], vmod_in, hs, "v")

    # ---- transpose v01m back to natural 96-pix tiles, stream out ----
    vT01f = vT01m.rearrange("p a t y x -> p a (t y x)")
    for t in range(8):  # (f, 2-row group)
        s = t * 96
        ps = ps_bf.tile([96, DIM], BF, tag="bf")
        for ic in range(NC4):
            nc.tensor.transpose(ps[:, ic * 128:(ic + 1) * 128],
                                vT01f[:, ic, s:s + 96], ident)
        vst = sb.tile([96, DIM], BF, tag="vst", name="vst")
        nc.vector.tensor_copy(vst, ps)
        _v_out(vst, 0, t)

    # ---- AG contributions ----
    kc = g["contrib"][0, OFF_K:OFF_V].rearrange("(a p t y x) -> p a t y x", a=NC4, p=128, t=T, y=STRIP_H)
    k01v = kT01m  # [128, 4, 2, 8, 48]
    k23v = kT23.rearrange("p a (t y x) -> p a t y x", t=2, y=STRIP_H)
    for ic in range(NC4):
        nc.sync.dma_start(kc[:, ic, 0:2, :, 0:48], k01v[:, ic])
        nc.sync.dma_start(kc[:, ic, 2:4, :, 0:48], k23v[:, ic])
        nc.sync.dma_start(kc[:, ic, 0:2, :, 48:64], k01v[:, ic, :, :, 0:16])
        nc.sync.dma_start(kc[:, ic, 2:4, :, 48:64], k23v[:, ic, :, :, 0:16])
    if "k_contrib" in g["dbg"]:
        nc.sync.dma_start(
            g["dbg"]["k_contrib"][:, :, :, :].rearrange("d t y x -> (d t y x)"),
            g["contrib"][0, OFF_K:OFF_V])
        nc.sync.dma_start(
            g["dbg"]["v_contrib"][:, :, :, :].rearrange("t y x d -> (t y x d)"),
            g["contrib"][0, OFF_V:OFF_PK])

    # ---- pooled tokens (72 rows on every core) ----
    px = sb.tile([PGRID, 6, DIM], BF)  # [cell 12, cell-row 6, ch]
    for cr in range(6):
        ps = ps_big.tile([PGRID, DIM], dt.float32, tag="big")
        for h in range(2):  # two 2-row groups of the 4-row cell
            xrows = sb.tile([96, DIM], BF, tag="xpoolrows")
            r0 = cr * 4 + h * 2
            nc.sync.dma_start(xrows, g["x_pool"][r0:r0 + 2].rearrange("y x c -> (y x) c"))
            nc.tensor.matmul(ps, pool_ind_t, xrows, start=(h == 0), stop=(h == 1))
        nc.vector.tensor_add(px[:, cr, :], ps, Bf["pool_b"][0:PGRID, :])
    # px^T [128,4,72]  (pooled-row order = (cell-row, cell), cr-major)
    pxT = sb.tile([128, NC4, POOL_PC], BF)
    for ic in range(NC4):
        ps = ps_bf.tile([128, POOL_PC], BF, tag="bf")
        for cr in range(6):
            nc.tensor.transpose(ps[:, cr * PGRID:(cr + 1) * PGRID],
                                px[:, cr, ic * 128:(ic + 1) * 128],
                                ident[0:PGRID, 0:PGRID])
        nc.vector.tensor_copy(pxT[:, ic, :], ps)
    # pk^T
    pkT = sb.tile([128, NC4, POOL_PC], BF)
    for oc in range(NC4):
        ps2 = ps_mid.tile([128, POOL_PC], dt.float32, tag="mid")
        for ic in range(NC4):
            nc.tensor.matmul(ps2, W["wk"][:, ic, oc * 128:(oc + 1) * 128],
                             pxT[:, ic, :], start=(ic == 0), stop=(ic == NC4 - 1))
        nc.scalar.activation(pkT[:, oc, :], ps2, AF.Identity, bias=Bp["bk"][:, oc:oc + 1])
    nc.sync.dma_start(g["contrib"][0, OFF_PK:OFF_PV].rearrange("(a p n) -> p a n", a=NC4, p=128), pkT)
    # pv natural
    pv = sb.tile([POOL_PC, DIM], BF)
    ps3 = ps_big.tile([POOL_PC, DIM], dt.float32, tag="big")
    for ic in range(NC4):
        nc.tensor.matmul(ps3, pxT[:, ic, 0:POOL_PC], W["wv"][:, ic, :],
                         start=(ic == 0), stop=(ic == NC4 - 1))
    nc.vector.tensor_add(pv, ps3, Bf["bv"][0:POOL_PC, :])
    nc.sync.dma_start(g["contrib"][0, OFF_PV:AG_TOT].rearrange("(n c) -> n c", n=POOL_PC), pv)
    if "pk" in g["dbg"]:
        nc.sync.dma_start(g["dbg"]["pk"][:, :].rearrange("d n -> (d n)"), g["contrib"][0, OFF_PK:OFF_PV])
        nc.sync.dma_start(g["dbg"]["pv"][:, :].rearrange("n d -> (n d)"), g["contrib"][0, OFF_PV:AG_TOT])


def _proj_T(nc, ps_pool, w, bias_p, xt, dst, npix, piece, tag="big"):
    """dst[:, oc, pix] = (W.T @ x^T)[oc chunk] + bias (transposed proj)."""
    n_p = (npix + piece - 1) // piece
    for oc in range(NC4):
        for p in range(n_p):
            s = p * piece
            e = min(npix, s + piece)
            ps = ps_pool.tile([128, piece], dt.float32, tag=tag)
            for ic in range(NC4):
                nc.tensor.matmul(ps[:, 0:e - s], w[:, ic, oc * 128:(oc + 1) * 128],
                                 xt[:, ic, s:e], start=(ic == 0), stop=(ic == NC4 - 1))
            nc.scalar.activation(dst[:, oc, s:e], ps[:, 0:e - s], AF.Identity,
                                 bias=bias_p[:, oc:oc + 1])


def _modulator(nc, tc, sb, ps_big, ps_mid, mw, mod_in, hs, tag):
    """Modulator on transposed f01 (k|v)+mask data [128, 4, 960] (10 rows
    incl halo). Returns modulated transposed [128, 4, 2, 8, 48]."""
    HP = STRIP_H + 2
    outT = sb.tile([128, NC4, 2, STRIP_H, WI], BF, tag=f"modo{tag}")
    with tc.tile_pool(name=f"sbM{tag}", bufs=1) as sbm:
        # conv1 1x1 512->128 (+bias) + leaky relu -> padded rows [128,2,10,50]
        lx1 = sbm.tile([128, 2, HP, 50], BF, tag="lx1")
        nc.vector.memset(lx1, 0.0)
        for f in range(2):
            s = f * (HP * WI)
            ps = ps_mid.tile([128, HP * WI], dt.float32, tag="mid")
            for ic in range(NC4):
                nc.tensor.matmul(ps, mw["sq"][:, ic, :],
                                 mod_in[:, ic, s:s + HP * WI],
                                 start=(ic == 0), stop=(ic == NC4 - 1))
            nc.scalar.activation(lx1[:, f, :, 1:49],
                                 ps.rearrange("p (y x) -> p y x", y=HP),
                                 AF.Identity, bias=mw["sqb"][:, 0:1])
            lint = lx1[:, f, :, 1:49]
            ltmp = sbm.tile([128, HP, 48], BF, tag="ltmp", name="ltmp")
            nc.vector.tensor_scalar_mul(ltmp, lint, 0.2)
            nc.vector.tensor_max(lint, lint, ltmp)
        # zero the halo rows at image edges (conv zero-padding semantics)
        for f in range(2):
            nc.vector.tensor_scalar_mul(lx1[:, f, 0, :], lx1[:, f, 0, :],
                                        hs[:, 0:1])
            nc.vector.tensor_scalar_mul(lx1[:, f, HP - 1, :],
                                        lx1[:, f, HP - 1, :], hs[:, 1:2])
        # conv2 3x3 128->128 (+bias) + SiLU -> lx2 [128, 2, 384]
        lx2 = sbm.tile([128, 2, STRIP_H * WI], BF, tag="lx2")
        for f in range(2):
            ps = ps_mid.tile([128, STRIP_H * WI], dt.float32, tag="mid")
            ti = 0
            for dy in (-1, 0, 1):
                for dx in (-1, 0, 1):
                    rhs = lx1[:, f, 1 + dy:1 + dy + STRIP_H, 1 + dx:1 + dx + WI]
                    nc.tensor.matmul(ps, mw["f"][:, ti, :], rhs,
                                     start=(ti == 0), stop=(ti == 8))
                    ti += 1
            sg = sbm.tile([128, STRIP_H * WI], BF, tag="modsg")
            nc.scalar.activation(sg, ps, AF.Sigmoid, bias=mw["fb"][:, 0:1])
            tmp = sbm.tile([128, STRIP_H * WI], BF, tag="modt")
            nc.scalar.activation(tmp, ps, AF.Identity, bias=mw["fb"][:, 0:1])
            nc.vector.tensor_mul(lx2[:, f, :], tmp, sg)
        # conv3 1x1 128->512 + bias -> transposed tile (strip rows 1..9)
        for f in range(2):
            for oc in range(NC4):
                ps = ps_mid.tile([128, STRIP_H * WI], dt.float32, tag="mid")
                nc.tensor.matmul(ps, mw["un"][:, oc * 128:(oc + 1) * 128],
                                 lx2[:, f, :], start=True, stop=True)
                nc.scalar.activation(outT[:, oc, f],
                                     ps.rearrange("p (y x) -> p y x", y=8),
                                     AF.Identity, bias=mw["unb"][:, oc:oc + 1])
    return outT


def _stage_b(nc, tc, g, sb0, ps_sc, ps_cmb, ps_dn, ps_y, W, Bf, qT,
             ones_col, ones_row):
    ag = g["agout"]
    kgv = ag[:, OFF_K:OFF_V].rearrange("s (a p t y x) -> p s a t y x",
                                       a=NC4, p=128, t=T, y=STRIP_H)
    vgv = ag[:, OFF_V:OFF_PK].rearrange("s (t y x c) -> s t y x c",
                                        t=T, y=STRIP_H, x=64)
    pkgv = ag[:, OFF_PK:OFF_PV].rearrange("s (a p n) -> p s a n", a=NC4, p=128)
    pvgv = ag[:, OFF_PV:AG_TOT].rearrange("s (n c) -> s n c", n=POOL_PC)

    with (
        tc.tile_pool(name="sbB", bufs=2) as sb,
        tc.tile_pool(name="sbBig", bufs=2) as sbig,
        tc.tile_pool(name="sbT", bufs=2) as sbT,
        tc.tile_pool(name="sbP", bufs=1) as sbP,
    ):
        # pooled keys, partition-packed into 4x128+64 chunks
        pk4 = sbP.tile([128, NC4, N_CORES, POOL_PC], BF)  # [ch, oc, core, n]
        for oc in range(NC4):
            nc.sync.dma_start(pk4[:, oc, :, :], pkgv[:, :, oc, :])
        pk_flat = pk4.rearrange("p a s n -> p a (s n)")  # [ch, oc, 576]
        pvc = sbP.tile([128, 5, DIM], BF)  # pooled v rows, 5 chunks
        for s in range(N_CORES):
            r = s * POOL_PC
            c0, off = r // 128, r % 128
            n1 = min(128 - off, POOL_PC)
            nc.sync.dma_start(pvc[off:off + n1, c0, :], pvgv[s, 0:n1, :])
            if n1 < POOL_PC:
                nc.sync.dma_start(pvc[0:POOL_PC - n1, c0 + 1, :],
                                  pvgv[s, n1:POOL_PC, :])

        # dynamic-offset registers (per window: prev,cur,next,x0,x_own)
        meta_t = sbP.tile([1, META_LEN], dt.int32)
        nc.sync.dma_start(meta_t, g["meta"][:].unsqueeze(0))
        regs = []
        for i in range(WPC * 5):
            r = nc.alloc_register(mybir.EngineType.SP, f"mreg{i}")
            nc.sync.reg_load(r, meta_t[0:1, i:i + 1])
            regs.append(nc.sync.snap(r))

        for wi in range(WPC):
            r_prev, r_cur, r_next, r_x0, r_xo = regs[wi * 5:wi * 5 + 5]
            ds = bass.ds
            # ---- k patch [128, 4(oc), 4(piece), 4(f), 4(y), 16(x)] ----
            kp = sbig.tile([128, NC4, 4, T, 4, 16], BF, tag="kp")
            pieces = [(r_prev, 4), (r_cur, 0), (r_cur, 4), (r_next, 0)]
            for oc in range(NC4):
                for pi, (slot, y0) in enumerate(pieces):
                    nc.sync.dma_start(
                        kp[:, oc, pi],
                        kgv[:, ds(slot, 1), oc, :, y0:y0 + 4, ds(r_x0, 16)]
                           .squeeze(1))
            # ---- v patch [128, 8, 512]: 16 (piece,f) blocks of 64 pix ----
            vp = sbig.tile([128, 8, DIM], BF, tag="vp")
            for pi, (slot, y0) in enumerate(pieces):
                for f in range(T):
                    b = pi * 4 + f
                    nc.sync.dma_start(
                        vp[64 * (b % 2):64 * (b % 2) + 64, b // 2, :],
                        vgv[ds(slot, 1), f, y0:y0 + 4, ds(r_x0, 16), :]
                           .squeeze(0))
            # ---- v own-window in rows 64:128 (aligns with local scores) ----
            vown = sbig.tile([128, T, DIM], BF, tag="vown")
            for f in range(T):
                nc.sync.dma_start(
                    vown[64:128, f, :],
                    vgv[ds(r_cur, 1), f, :, ds(r_xo, 8), :]
                       .squeeze(0))
            # ---- k^T own-window [128, 4(oc), 4(f), 64] ----
            kown = sbig.tile([128, NC4, T, WN], BF, tag="kown")
            for oc in range(NC4):
                nc.sync.dma_start(
                    kown[:, oc].rearrange("p f (y x) -> p f y x", y=WH),
                    kgv[:, ds(r_cur, 1), oc, :, :, ds(r_xo, 8)].squeeze(1))
            # ---- flag bcast [128,1] ----
            flb = sb.tile([128, 1], dt.float32, tag="flb")
            nc.sync.dma_start(flb, g["flags_d"][wi:wi + 1].unsqueeze(0)
                              .to_broadcast([128, 1]))

            q_w = qT[:, :, wi * NQ:(wi + 1) * NQ]  # [128, 4, 256]
            kpf = kp.rearrange("p a x t y xx -> p a (x t y xx)")  # [128,4,1024]
            yfin = sbig.tile([128, N_HEAD, NQ], BF, tag="yfin")

            for h in range(N_HEAD):
                # ---------- global QK + exp (groups of GSZ chunks) --------
                pT = sbT.tile([128, NGRP * GSZ, NQ], BF, tag="pT")
                pTc = sbT.tile([128, NQ], BF, tag="pTc")
                den = ps_dn.tile([33, NQ], dt.float32, tag="dn")
                for grp in range(NGRP):
                    sc = ps_sc.tile([128, GSZ * NQ], dt.float32, tag="sc")
                    for jj in range(GSZ):
                        j = grp * GSZ + jj
                        if j < 8:
                            lhs = kpf[:, h, j * 128:(j + 1) * 128]
                        else:
                            lhs = pk_flat[:, h, (j - 8) * 128:(j - 7) * 128]
                        nc.tensor.matmul(sc[:, jj * NQ:(jj + 1) * NQ], lhs,
                                         q_w[:, h, :], start=True, stop=True)
                    nc.scalar.activation(
                        pT[:, grp * GSZ:(grp + 1) * GSZ, :].rearrange("p a b -> p (a b)"),
                        sc, AF.Exp, scale=SCALE)
                # chunk 12 (64 pool keys, rows 0:64) + local scores (64:128)
                cmb = ps_cmb.tile([128, NQ], dt.float32, tag="cmb")
                nc.tensor.matmul(cmb[0:64, :], pk_flat[:, h, 512:576],
                                 q_w[:, h, :], start=True, stop=True)
                for f in range(T):
                    nc.tensor.matmul(
                        cmb[64:128, f * WN:(f + 1) * WN],
                        kown[:, h, f, :], q_w[:, h, f * WN:(f + 1) * WN],
                        start=True, stop=True)
                nc.scalar.activation(pTc, cmb, AF.Exp, scale=SCALE)
                # ---------- denominators on PE (ones-vector matmuls) ------
                for j in range(12):
                    nc.tensor.matmul(den[0:1, :], ones_col[:, 0:1],
                                     pT[:, j, :], start=(j == 0), stop=False)
                nc.tensor.matmul(den[0:1, :], ones_col[0:64, 0:1],
                                 pTc[0:64, :], start=False, stop=True)
                nc.tensor.matmul(den[32:33, :], ones_col[64:128, 0:1],
                                 pTc[64:128, :], start=True, stop=True)
                rec = sb.tile([33, NQ], BF, tag="rec")
                with nc.allow_low_precision(reason="softmax recip row, 0.4% rel"):
                    nc.vector.reciprocal(rec[0:1, :], den[0:1, :])
                    nc.vector.reciprocal(rec[32:33, :], den[32:33, :])
                # ---------- AV ----------
                psy = ps_y.tile([128, NQ], dt.float32, tag="y")
                for j in range(NCHUNK):
                    if j < 8:
                        lhs = vp[0:128, j, h * 128:(h + 1) * 128]
                        rhs = pT[:, j, :]
                    elif j < 12:
                        lhs = pvc[:, j - 8, h * 128:(h + 1) * 128]
                        rhs = pT[:, j, :]
                    else:
                        lhs = pvc[0:64, 4, h * 128:(h + 1) * 128]
                        rhs = pTc[0:64, :]
                    nc.tensor.matmul(psy, lhs, rhs,
                                     start=(j == 0), stop=(j == NCHUNK - 1))
                # local AV: per-frame over own-window keys (rows 64:128)
                psyl = ps_y.tile([128, NQ], dt.float32, tag="y")
                for f in range(T):
                    nc.tensor.matmul(psyl[:, f * WN:(f + 1) * WN],
                                     vown[64:128, f, h * 128:(h + 1) * 128],
                                     pTc[64:128, f * WN:(f + 1) * WN],
                                     start=True, stop=True)
                # ---------- normalize (PE broadcast of recip) + blend -----
                bc = ps_dn.tile([128, 2, NQ], dt.float32, tag="dn")
                nc.tensor.matmul(bc[:, 0, :], ones_row[0:1, :], rec[0:1, :],
                                 start=True, stop=True)
                nc.tensor.matmul(bc[:, 1, :], ones_row[32:33, :], rec[32:33, :],
                                 start=True, stop=True)
                bcs = sb.tile([128, 2, NQ], BF, tag="bcs")
                nc.vector.tensor_copy(bcs, bc)
                y_g = sb.tile([128, NQ], BF, tag="yg_sb")
                nc.vector.tensor_mul(y_g, psy, bcs[:, 0, :])
                y_l = sb.tile([128, NQ], BF, tag="yl_sb")
                nc.vector.tensor_mul(y_l, psyl, bcs[:, 1, :])
                # y = y_l + flag*(y_g - y_l)
                dlt = sb.tile([128, NQ], BF, tag="dlt")
                nc.vector.tensor_sub(dlt, y_g, y_l)
                nc.vector.tensor_scalar_mul(dlt, dlt, flb[:, 0:1])
                nc.vector.tensor_add(yfin[:, h, :], y_l, dlt)

            # ================= output projection =================
            for fp in range(2):
                for half in range(2):
                    pso = ps_sc.tile([128, NQ], dt.float32, tag="sc")
                    for h in range(N_HEAD):
                        nc.tensor.matmul(
                            pso, yfin[:, h, fp * 128:(fp + 1) * 128],
                            W["wp"][:, h, half * 256:(half + 1) * 256],
                            start=(h == 0), stop=(h == N_HEAD - 1))
                    osb = sb.tile([128, NQ], dt.float32, tag="osb")
                    nc.vector.tensor_add(osb, pso,
                                         Bf["bp"][:, half * 256:(half + 1) * 256])
                    nc.sync.dma_start(
                        g["out_win"][wi, 2 * fp:2 * fp + 2, :,
                                     half * 256:(half + 1) * 256]
                            .rearrange("t p c -> (t p) c"), osb)


# ==================== host side ====================

def _host_inputs(inputs, debug=False):
    x = np.asarray(inputs["x"], np.float32)[0]  # [4,48,48,512]
    mask = np.asarray(inputs["mask"], np.float32)[0, :, :, :, 0]  # [2,48,48]
    xb = x.astype(NPBF)

    pool_ind = np.zeros((2 * WI, PGRID), np.float32)
    for y in range(2):
        for xx in range(WI):
            pool_ind[y * WI + xx, xx // PW] = 1.0 / (PH * PW)

    common = dict(
        wq=np.asarray(inputs["Wq"], np.float32).astype(NPBF),
        bq=np.asarray(inputs["bq"], np.float32),
        wk=np.asarray(inputs["Wk"], np.float32).astype(NPBF),
        bk=np.asarray(inputs["bk"], np.float32),
        wv=np.asarray(inputs["Wv"], np.float32).astype(NPBF),
        bv=np.asarray(inputs["bv"], np.float32),
        wp=np.asarray(inputs["Wp"], np.float32).astype(NPBF),
        bp=np.asarray(inputs["bp"], np.float32),
        pool_b=np.asarray(inputs["pool_b"], np.float32),
        pool_ind=pool_ind.astype(NPBF),
    )
    for tag, pre in (("k", "kmod"), ("v", "vmod")):
        common[f"{tag}sq"] = np.ascontiguousarray(
            np.asarray(inputs[f"{pre}_sq_w"], np.float32)[:, :, 0, 0].T).astype(NPBF)
        common[f"{tag}sqb"] = np.asarray(inputs[f"{pre}_sq_b"], np.float32)
        fw = np.asarray(inputs[f"{pre}_f_w"], np.float32)
        common[f"{tag}f"] = np.ascontiguousarray(
            np.stack([fw[:, :, dy, dx].T for dy in range(3) for dx in range(3)])).astype(NPBF)
        common[f"{tag}fb"] = np.asarray(inputs[f"{pre}_f_b"], np.float32)
        common[f"{tag}un"] = np.ascontiguousarray(
            np.asarray(inputs[f"{pre}_un_w"], np.float32)[:, :, 0, 0].T).astype(NPBF)
        common[f"{tag}unb"] = np.asarray(inputs[f"{pre}_un_b"], np.float32)

    in_maps = []
    for c in range(N_CORES):
        m = dict(common)
        # strip rows with halo
        if c < N_STRIP:
            r0 = c * STRIP_H
            xs = np.zeros((T, STRIP_H + 2, WI, DIM), NPBF)
            ms = np.zeros((L_T, STRIP_H + 2, WI), NPBF)
            lo, hi = max(0, r0 - 1), min(HI, r0 + STRIP_H + 1)
            xs[:, lo - (r0 - 1):lo - (r0 - 1) + hi - lo] = xb[:, lo:hi]
            ms[:, lo - (r0 - 1):lo - (r0 - 1) + hi - lo] = \
                mask[:, lo:hi].astype(NPBF)
            m["x_strip"] = xs
            m["mask_strip"] = ms
            m["halo_scale"] = np.array(
                [0.0 if r0 == 0 else 1.0,
                 0.0 if r0 + STRIP_H == HI else 1.0], np.float32)
        else:
            m["x_strip"] = np.zeros((T, STRIP_H + 2, WI, DIM), NPBF)
            m["mask_strip"] = np.zeros((L_T, STRIP_H + 2, WI), NPBF)
            m["halo_scale"] = np.ones(2, np.float32)
        # window inputs
        xw = np.zeros((WPC, T, WN, DIM), NPBF)
        mw = np.zeros((L_T, WPC, WN), np.float32)
        for k, w in enumerate(WIN_ASSIGN[c]):
            i, j = w // N_WW, w % N_WW
            blk = xb[:, 8 * i:8 * i + 8, 8 * j:8 * j + 8, :]
            xw[k] = blk.reshape(T, WN, DIM)
            mw[:, k] = mask[:, 8 * i:8 * i + 8, 8 * j:8 * j + 8].reshape(L_T, WN)
        m["x_win"] = xw
        m["mask_win"] = mw
        # pool rows: 72 pooled cells = frame c//2, cell-rows 6*(c%2)..+6
        f, pr0 = c // 2, 6 * (c % 2)
        m["x_pool"] = np.ascontiguousarray(xb[f, pr0 * 4:pr0 * 4 + 24])
        m["meta"] = _meta_for_core(c)
        in_maps.append(m)
    return in_maps


def _get_nc(debug=False):
    key = bool(debug)
    if key not in _NC_CACHE:
        _NC_CACHE[key] = build_nc(debug=debug)
    return _NC_CACHE[key]


def run_spmd(inputs, debug=False):
    nc = _get_nc(debug=debug)
    in_maps = _host_inputs(inputs, debug=debug)
    res = run_bass_kernel_spmd(nc, in_maps, list(range(N_CORES)))
    return res


def assemble(results):
    out = np.zeros((T, HI, WI, DIM), np.float32)
    done = set()
    for c in range(N_CORES):
        ow = np.asarray(results[c]["out_win"], np.float32)  # [5,4,64,512]
        for k, w in enumerate(WIN_ASSIGN[c]):
            if w in done:
                continue
            done.add(w)
            i, j = w // N_WW, w % N_WW
            out[:, 8 * i:8 * i + 8, 8 * j:8 * j + 8, :] = \
                ow[k].reshape(T, WH, WW, DIM)
    return out[None]


def kernel(**inputs):
    res = run_spmd(inputs)
    return assemble(res.results)


_CALLABLE_CACHE = {}


def _get_callable(debug=False):
    """Build the sharded jitted callable once (mirrors run_bass_via_pjrt)."""
    key = bool(debug)
    if key in _CALLABLE_CACHE:
        return _CALLABLE_CACHE[key]
    import jax
    from jax.sharding import Mesh, PartitionSpec
    from jax.experimental.shard_map import shard_map
    from concourse import bass2jax, mybir as _mb

    nc = _get_nc(debug=debug)
    bass2jax.install_neuronx_cc_hook()
    in_names, out_names, out_avals, zero_outs = [], [], [], []
    pname = nc.partition_id_tensor.name if nc.partition_id_tensor else None
    for alloc in nc.m.functions[0].allocations:
        if not isinstance(alloc, _mb.MemoryLocationSet):
            continue
        name = alloc.memorylocations[0].name
        if alloc.kind == "ExternalInput":
            if name != pname:
                in_names.append(name)
        elif alloc.kind == "ExternalOutput":
            out_names.append(name)
            shape = tuple(alloc.tensor_shape)
            dtp = _mb.dt.np(alloc.dtype)
            out_avals.append(jax.core.ShapedArray(shape, dtp))
            zero_outs.append(np.zeros(shape, dtp))
    n_params = len(in_names)
    all_in = list(in_names) + list(out_names)
    if pname is not None:
        all_in.append(pname)

    def _body(*args):
        ops = list(args)
        if pname is not None:
            ops.append(bass2jax.partition_id_tensor())
        return tuple(bass2jax._bass_exec_p.bind(
            *ops, out_avals=tuple(out_avals), in_names=tuple(all_in),
            out_names=tuple(out_names), lowering_input_output_aliases=(),
            sim_require_finite=True, sim_require_nnan=True, nc=nc))

    devices = jax.devices()[:N_CORES]
    mesh = Mesh(np.asarray(devices), ("core",))
    n_outs = len(out_names)
    sharded = jax.jit(
        shard_map(_body, mesh=mesh,
                  in_specs=(PartitionSpec("core"),) * (n_params + n_outs),
                  out_specs=(PartitionSpec("core"),) * n_outs,
                  check_rep=False),
        donate_argnums=tuple(range(n_params, n_params + n_outs)),
        keep_unused=True)
    info = (sharded, in_names, out_names, out_avals, zero_outs)
    _CALLABLE_CACHE[key] = info
    return info


def timed_run(inputs, iters=4, debug=False):
    """Run via a cached jitted callable; returns (results, best_wall_s)."""
    import time as _time
    import jax
    sharded, in_names, out_names, out_avals, zero_outs = _get_callable(debug)
    in_maps = _host_inputs(inputs, debug=debug)
    dbgz = np.zeros((1, 2), np.uint32)  # dbg_addr placeholder (debug builds)
    concat_in = [np.concatenate(
        [np.asarray(in_maps[c].get(n, dbgz)) for c in range(N_CORES)], 0)
        for n in in_names]
    concat_in = [jax.device_put(a) for a in concat_in]
    best = None
    out_arrs = None
    for _ in range(iters):
        zeros = [np.zeros((N_CORES * z.shape[0],) + z.shape[1:], z.dtype)
                 for z in zero_outs]
        t0 = _time.perf_counter()
        out_arrs = sharded(*concat_in, *zeros)
        jax.block_until_ready(out_arrs)
        dt_ = _time.perf_counter() - t0
        best = dt_ if best is None else min(best, dt_)
    results = [
        {n: np.asarray(out_arrs[i]).reshape(N_CORES, *out_avals[i].shape)[c]
         for i, n in enumerate(out_names)}
        for c in range(N_CORES)
    ]
    return results, best
